# revision 13
# baseline (speedup 1.0000x reference)
"""Bone_Direction_GCN fused kernel, RPP=4 layout, for 8 Trainium2 NeuronCores.

Same math as kernel.py but each SBUF partition holds 4 CONSECUTIVE rows
(rows 4p+w, w in 0..3), so every HBM DMA descriptor covers 2 KB instead of
512 B (4x fewer descriptors; HW DMA is descriptor-fixed-cost bound).

The graph mix becomes 4 accumulating matmuls (one per row-in-partition slot w
of the source), x^T is recovered with 4 PE transposes, and the adj-mix runs
as 16 small matmuls whose host-built constants absorb the row permutation.
"""

import sys

sys.path.insert(0, "/opt/trn_rl_repo")

import numpy as np
import ml_dtypes

B, J, E = 16384, 17, 32
CIN, COUT = 128, 128
MID = COUT // 2
PROP = 0.5
SLOPE = 0.01

N_CORES = 8
BC = B // N_CORES          # batches per core (2048)
ROWS = BC * J              # rows per core (34816)
W = 4                      # rows per partition
P = 119                    # partitions used per macro-tile
RM = W * P                 # rows per macro-tile (476)
NB = RM // J               # batches per macro-tile (28)
LM = 4                     # macro-tiles per full DMA group
GSIZES = [4] * 18 + [1]    # 18 groups of 4 + final group of 1 (73 macros)
NG = len(GSIZES)
NMM = 73                   # all macro-tiles pipelined
NM = 73                    # total macro-tiles (73*476 = 34748)
GT = BC - NM * NB          # tail batches (4)
RT = GT * J                # tail rows (68)
R1 = 119                   # legacy sub-tile rows (epilogue/tail path)
PB = 120                   # padded transpose block (alignment)
CPAD = 256

assert NM * RM + RT == ROWS

_CACHE = {}


def _gcn_matrix(edge_index, edge_weight):
    row = edge_index[0].astype(np.int64)
    col = edge_index[1].astype(np.int64)
    loop = np.arange(J, dtype=np.int64)
    row_f = np.concatenate([row, loop])
    col_f = np.concatenate([col, loop])
    w_f = np.concatenate([edge_weight.astype(np.float32), np.ones(J, np.float32)])
    deg = np.zeros(J, np.float32)
    np.add.at(deg, col_f, w_f)
    safe = np.where(deg > 0, deg, 1.0).astype(np.float32)
    dis = np.where(deg > 0, 1.0 / np.sqrt(safe), 0.0).astype(np.float32)
    norm = dis[row_f] * w_f * dis[col_f]
    M = np.zeros((J, J), np.float32)
    np.add.at(M, (col_f, row_f), norm)
    return M


def _block_diag(block, n):
    j = block.shape[0]
    out = np.zeros((n * j, n * j), block.dtype)
    for g in range(n):
        out[g * j:(g + 1) * j, g * j:(g + 1) * j] = block
    return out


def _mix_consts_legacy(M, adj, g):
    r = g * J
    mix1 = _block_diag(M.T, g)
    mixI = np.concatenate([mix1, np.eye(r, dtype=np.float32)], axis=1)
    mix2 = _block_diag(PROP * adj, g)
    ones_row = np.ones((1, r), np.float32)
    s_row = np.tile(PROP * adj.sum(axis=0), g)[None, :]
    mix2e = np.concatenate([mix2, ones_row, s_row], axis=0)
    return mixI, mix2e


def _rpp4_consts(M, adj):
    """mixu [119, 4, 476]; m2 [121, 4, 4, 119] for the w/u-blocked adj mix."""
    Mblk = _block_diag(M.T, NB)                    # [476, 476]: rows=src, cols=dst
    mixu = np.zeros((P, W, RM), np.float32)
    for p in range(P):
        for u in range(W):
            mixu[p, u, :] = Mblk[W * p + u, :]
    adjm = PROP * adj                              # coeff d[v] -> out[w]: adjm[v, w]
    s_row_j = PROP * adj.sum(axis=0)               # [J]
    m2 = np.zeros((P + 2, W, W, P), np.float32)
    for pd in range(P):                            # y2e partition (d row 4*pd+u)
        for u in range(W):
            q = W * pd + u
            for po in range(P):                    # out partition (row 4*po+w)
                for w in range(W):
                    r = W * po + w
                    if q // J == r // J:
                        m2[pd, w, u, po] = adjm[q % J, r % J]
    for po in range(P):
        for w in range(W):
            r = W * po + w
            m2[P, w, 0, po] = 1.0                  # b1 row
            m2[P + 1, w, 0, po] = s_row_j[r % J]   # b4 row
    return mixu.reshape(P, W * RM), m2.reshape(P + 2, W * W * P)


def _build_bass(leaky_mode: str = "lrelu", **_ignored):
    import concourse.bacc as bacc
    import concourse.mybir as mybir
    import concourse.tile as tile
    from contextlib import ExitStack

    f32 = mybir.dt.float32
    bf16 = mybir.dt.bfloat16

    nc = bacc.Bacc("TRN2", target_bir_lowering=False, debug=False)

    x_d = nc.dram_tensor("x", [ROWS, CIN], f32, kind="ExternalInput").ap()
    mixu_d = nc.dram_tensor("mixu", [P, W * RM], bf16, kind="ExternalInput").ap()
    m2_d = nc.dram_tensor("m2", [P + 2, W * W * P], bf16, kind="ExternalInput").ap()
    ident_d = nc.dram_tensor("ident", [PB, PB], bf16, kind="ExternalInput").ap()
    mixIt_d = nc.dram_tensor("mixIt", [RT, CPAD], bf16, kind="ExternalInput").ap()
    mix2et_d = nc.dram_tensor("mix2et", [RT + 2, RT], bf16, kind="ExternalInput").ap()
    w1_d = nc.dram_tensor("w1", [CIN, COUT], bf16, kind="ExternalInput").ap()
    w2t_d = nc.dram_tensor("w2t", [CIN, MID], bf16, kind="ExternalInput").ap()
    w4t_d = nc.dram_tensor("w4t", [MID, COUT], bf16, kind="ExternalInput").ap()
    b2_d = nc.dram_tensor("b2", [MID, 1], f32, kind="ExternalInput").ap()
    ab2_d = nc.dram_tensor("ab2", [MID, 1], f32, kind="ExternalInput").ap()
    b1b4_d = nc.dram_tensor("b1b4", [2, W * COUT], bf16, kind="ExternalInput").ap()
    o_d = nc.dram_tensor("out", [ROWS, CIN], f32, kind="ExternalOutput").ap()

    with ExitStack() as ctx:
        tc = ctx.enter_context(tile.TileContext(nc))

        const = ctx.enter_context(tc.tile_pool(name="const", bufs=1))
        mixu_sb = const.tile_from(mixu_d)
        m2_sb = const.tile_from(m2_d)
        ident_sb = const.tile_from(ident_d)
        mixIt_sb = const.tile_from(mixIt_d)
        mix2et_sb = const.tile_from(mix2et_d)
        w1_sb = const.tile_from(w1_d)
        w2t_sb = const.tile_from(w2t_d)
        w4t_sb = const.tile_from(w4t_d)
        b2_sb = const.tile_from(b2_d)
        ab2_sb = const.tile_from(ab2_d)

        def leaky(hbf, psH):
            if leaky_mode == "lrelu":
                nc.scalar.activation(
                    hbf[:], psH[:],
                    func=mybir.ActivationFunctionType.Lrelu,
                    bias=b2_sb[:], scale=1.0, alpha=SLOPE,
                )
            else:
                a = lk_pool.tile(list(psH.shape), bf16, tag="lk_a", name="lk_a")
                nc.scalar.activation(
                    a[:], psH[:],
                    func=mybir.ActivationFunctionType.Identity,
                    bias=ab2_sb[:], scale=SLOPE,
                )
                nc.vector.scalar_tensor_tensor(
                    hbf[:], psH[:], b2_sb[:], a[:],
                    op0=mybir.AluOpType.add, op1=mybir.AluOpType.max,
                )

        y2e_pool = ctx.enter_context(tc.tile_pool(name="y2e", bufs=3))
        y2e_tiles = []
        for i in range(3):
            t = y2e_pool.tile([P + 2, W * COUT], bf16, tag=f"y2e{i}", name=f"y2e{i}")
            nc.sync.dma_start(out=t[P:P + 2, :], in_=b1b4_d)
            y2e_tiles.append(t)
        y2et_pool = ctx.enter_context(tc.tile_pool(name="y2et", bufs=1))
        y2et = y2et_pool.tile([RT + 2, COUT], bf16)
        nc.sync.dma_start(out=y2et[RT:RT + 2, :], in_=b1b4_d[:, 0:COUT])

        lk_pool = ctx.enter_context(tc.tile_pool(name="lk", bufs=2))

        xin_pool = ctx.enter_context(tc.tile_pool(name="xin", bufs=3))
        xbf_pool = ctx.enter_context(tc.tile_pool(name="xbf", bufs=3))
        xbf_ring = []
        for i in range(3):
            t = xbf_pool.tile([PB, W * CIN], bf16, tag=f"xbfr{i}", name=f"xbfr{i}")
            nc.gpsimd.memset(t[:], 0.0)
            xbf_ring.append(t)
        ogrp_pool = ctx.enter_context(tc.tile_pool(name="ogrp", bufs=3))
        xmxt_pool = ctx.enter_context(tc.tile_pool(name="xmxt", bufs=4))
        h_pool = ctx.enter_context(tc.tile_pool(name="h", bufs=3))

        psTm_pool = ctx.enter_context(tc.tile_pool(name="psTm", bufs=2, space="PSUM"))
        psTr_pool = ctx.enter_context(tc.tile_pool(name="psTr", bufs=2, space="PSUM"))
        psH_pool = ctx.enter_context(tc.tile_pool(name="psH", bufs=1, space="PSUM"))
        psY2_pool = ctx.enter_context(tc.tile_pool(name="psY2", bufs=1, space="PSUM"))
        psO_pool = ctx.enter_context(tc.tile_pool(name="psO", bufs=2, space="PSUM"))

        xin_tiles = [None] * NG
        ogrp_tiles = [None] * NG
        g_of, k_of, g_row = [], [], []
        r_acc = 0
        for g, sz in enumerate(GSIZES):
            g_row.append(r_acc)
            for k in range(sz):
                g_of.append(g)
                k_of.append(k)
            r_acc += sz * RM
        xbf_tiles = [None] * NMM
        psTm_tiles = [None] * NMM
        psTr_tiles = [None] * NMM
        xm_tiles = [None] * NMM
        xt_tiles = [None] * NMM
        psH_tiles = [None] * NMM
        psY2_tiles = [None] * NMM
        psO_tiles = [None] * NMM
        hbf_tiles = [None] * NMM

        MFREE = W * CIN            # 512 f32 per partition per macro

        def xin_slice(m):
            g, k = g_of[m], k_of[m]
            return xin_tiles[g][:, k * MFREE:(k + 1) * MFREE]

        def stage_load(g):
            r0 = g_row[g]
            sz = GSIZES[g]
            t = xin_pool.tile([P, sz * MFREE], f32, tag="xin", name="xin")
            xin_tiles[g] = t
            nc.sync.dma_start(
                out=t[:].rearrange("p (k q) -> p k q", q=MFREE),
                in_=x_d[r0:r0 + sz * RM, :].rearrange(
                    "(k p w) c -> p k (w c)", p=P, w=W),
            )
            ogrp_tiles[g] = ogrp_pool.tile(
                [P, sz * MFREE], f32, tag="ogrp", name="ogrp")

        def stage_cast(m):
            t = xbf_ring[m % 3]
            xbf_tiles[m] = t
            nc.gpsimd.tensor_copy(t[0:P, :], xin_slice(m))

        def stage_mix(m):
            xbf = xbf_tiles[m]
            psTm = psTm_pool.tile([CIN, RM], f32, tag="psTm", name="psTm")
            psTm_tiles[m] = psTm
            for u in range(W):
                nc.tensor.matmul(
                    psTm[:],
                    lhsT=xbf[0:P, u * CIN:(u + 1) * CIN],
                    rhs=mixu_sb[:].rearrange("p (u n) -> p u n", u=W)[:, u, :],
                    start=(u == 0), stop=(u == W - 1),
                )
            psTr = psTr_pool.tile([CIN, W * PB], bf16, tag="psTr", name="psTr")
            psTr_tiles[m] = psTr
            for u in range(W):
                nc.tensor.transpose(
                    psTr[:, u * PB:(u + 1) * PB],
                    in_=xbf[0:PB, u * CIN:(u + 1) * CIN],
                    identity=ident_sb[:],
                )

        def stage_copies(m):
            # xm: [128, (w p)] bf16 (w-major so W1 lhsT slices are contiguous)
            xm = xmxt_pool.tile([CIN, RM], bf16, tag="xm", name="xm")
            xm_tiles[m] = xm
            nc.vector.tensor_copy(
                xm[:].rearrange("c (w p) -> c w p", w=W),
                psTm_tiles[m][:].rearrange("c (p w) -> c w p", w=W),
            )
            xt = xmxt_pool.tile([CIN, W * PB], bf16, tag="xt", name="xt")
            xt_tiles[m] = xt
            nc.scalar.copy(xt[:], psTr_tiles[m][:])

        def stage_w2(m):
            psH = psH_pool.tile([MID, W * PB], f32, tag="psH", name="psH")
            psH_tiles[m] = psH
            nc.tensor.matmul(
                psH[:], lhsT=w2t_sb[:], rhs=xt_tiles[m][:], start=True, stop=True)
            hbf = h_pool.tile([MID, W * PB], bf16, tag="hbf", name="hbf")
            hbf_tiles[m] = hbf
            leaky(hbf, psH)

        def stage_w4(m):
            hbf = hbf_tiles[m]
            psY2 = psY2_pool.tile([P, W * COUT], f32, tag="psY2", name="psY2")
            psY2_tiles[m] = psY2
            for u in range(W):
                nc.tensor.matmul(
                    psY2[:, u * COUT:(u + 1) * COUT],
                    lhsT=hbf[:, u * PB:u * PB + P], rhs=w4t_sb[:],
                    start=True, stop=True,
                )

        def stage_y2e(m):
            psY2 = psY2_tiles[m]
            y2e = y2e_tiles[m % 3]
            nc.vector.tensor_copy(y2e[0:P, 0:2 * COUT], psY2[:, 0:2 * COUT])
            nc.scalar.copy(y2e[0:P, 2 * COUT:], psY2[:, 2 * COUT:])

        def stage_out(m):
            xm = xm_tiles[m]
            y2e = y2e_tiles[m % 3]
            m2v = m2_sb[:].rearrange("p (w u q) -> p w u q", w=W, u=W)
            psO = psO_pool.tile([P, W * COUT], f32, tag="psO", name="psO")
            psO_tiles[m] = psO
            for w in range(W):
                nc.tensor.matmul(
                    psO[:, w * COUT:(w + 1) * COUT],
                    lhsT=xm[:, w * P:(w + 1) * P], rhs=w1_sb[:],
                    start=True, stop=False, skip_group_check=True,
                )
                for u in range(W):
                    nc.tensor.matmul(
                        psO[:, w * COUT:(w + 1) * COUT],
                        lhsT=m2v[:, w, u, :],
                        rhs=y2e[:, u * COUT:(u + 1) * COUT],
                        start=False, stop=(u == W - 1), skip_group_check=True,
                    )

        def stage_add(m):
            g, k = g_of[m], k_of[m]
            nc.vector.tensor_add(
                ogrp_tiles[g][:, k * MFREE:(k + 1) * MFREE],
                psO_tiles[m][:], xin_slice(m))

        def stage_store(g):
            r0 = g_row[g]
            sz = GSIZES[g]
            nc.sync.dma_start(
                out=o_d[r0:r0 + sz * RM, :].rearrange(
                    "(k p w) c -> p k (w c)", p=P, w=W),
                in_=ogrp_tiles[g][:].rearrange("p (k q) -> p k q", q=MFREE),
            )

        for it in range(NMM + 3):
            m1, m2i, m3 = it - 1, it - 2, it - 3
            if it < NMM and k_of[it] == 0:
                stage_load(g_of[it])
            if it < NMM:
                stage_cast(it)
            if 0 <= m3 < NMM:
                stage_y2e(m3)
            if 0 <= m1 < NMM:
                stage_mix(m1)
            if 0 <= m2i < NMM:
                stage_w2(m2i)
            if 0 <= m1 < NMM:
                stage_copies(m1)
            if 0 <= m3 < NMM:
                stage_out(m3)
                stage_add(m3)
            if 0 <= m2i < NMM:
                stage_w4(m2i)
            if 0 <= m3 < NMM and k_of[m3] == GSIZES[g_of[m3]] - 1:
                stage_store(g_of[m3])

        # ---- tail (68 rows): legacy s-major path ----
        r0 = NM * RM
        xin = xin_pool.tile([RT, CIN], f32, tag="xin_t")
        nc.sync.dma_start(out=xin[:], in_=x_d[r0:r0 + RT, :])
        xbf = xbf_pool.tile([RT, CIN], bf16, tag="xbf_t")
        nc.gpsimd.tensor_copy(xbf[:], xin[:])
        psT = psTm_pool.tile([CIN, CPAD], f32, tag="psTm", name="psT_t")
        nc.tensor.matmul(psT[:], lhsT=xbf[:], rhs=mixIt_sb[:], start=True, stop=True)
        xm = xmxt_pool.tile([CIN, RT], bf16, tag="xm_t")
        nc.vector.tensor_copy(xm[:], psT[:, 0:RT])
        xt = xmxt_pool.tile([CIN, RT], bf16, tag="xt_t")
        nc.scalar.copy(xt[:], psT[:, RT:2 * RT])
        psH = psH_pool.tile([MID, RT], f32, tag="psH")
        nc.tensor.matmul(psH[:], lhsT=w2t_sb[:], rhs=xt[:], start=True, stop=True)
        hbf = h_pool.tile([MID, RT], bf16, tag="hbf")
        leaky(hbf, psH)
        psY2 = psY2_pool.tile([RT, COUT], f32, tag="psY2")
        nc.tensor.matmul(psY2[:], lhsT=hbf[:], rhs=w4t_sb[:], start=True, stop=True)
        nc.scalar.copy(y2et[0:RT, :], psY2[:])
        psO = psO_pool.tile([RT, COUT], f32, tag="psO")
        nc.tensor.matmul(psO[:], lhsT=xm[:], rhs=w1_sb[:],
                         start=True, stop=False, skip_group_check=True)
        nc.tensor.matmul(psO[:], lhsT=mix2et_sb[:], rhs=y2et[:],
                         start=False, stop=True, skip_group_check=True)
        out_sb = xin_pool.tile([RT, CIN], f32, tag="out_t")
        nc.vector.tensor_add(out_sb[:], psO[:], xin[:])
        nc.sync.dma_start(out=o_d[r0:r0 + RT, :], in_=out_sb[:])

    nc.compile()
    return nc


def _host_consts(inputs):
    bf = ml_dtypes.bfloat16
    M = _gcn_matrix(np.asarray(inputs["edge_index"]), np.asarray(inputs["edge_weight"]))
    adj = np.asarray(inputs["adj"], np.float32)
    mixu, m2 = _rpp4_consts(M, adj)
    mixIt, mix2et = _mix_consts_legacy(M, adj, GT)
    mixIt_p = np.zeros((RT, CPAD), np.float32)
    mixIt_p[:, :2 * RT] = mixIt
    W1 = np.asarray(inputs["W1"], np.float32)
    W2 = np.asarray(inputs["W2"], np.float32)
    W4 = np.asarray(inputs["W4"], np.float32)
    b1 = np.asarray(inputs["b1"], np.float32)
    b2 = np.asarray(inputs["b2"], np.float32)
    b4 = np.asarray(inputs["b4"], np.float32)
    b1b4 = np.stack([np.tile(b1, W), np.tile(b4, W)])
    return {
        "mixu": mixu.astype(bf),
        "m2": m2.astype(bf),
        "ident": np.eye(PB, dtype=np.float32).astype(bf),
        "mixIt": mixIt_p.astype(bf),
        "mix2et": mix2et.astype(bf),
        "w1": np.ascontiguousarray(W1).astype(bf),
        "w2t": np.ascontiguousarray(W2.T).astype(bf),
        "w4t": np.ascontiguousarray(W4.T).astype(bf),
        "b2": np.ascontiguousarray(b2[:, None]),
        "ab2": np.ascontiguousarray(SLOPE * b2[:, None]),
        "b1b4": b1b4.astype(bf),
    }


def kernel(**inputs) -> np.ndarray:
    from concourse.bass_utils import run_bass_kernel_spmd

    if "nc" not in _CACHE:
        _CACHE["nc"] = _build_bass()
    nc = _CACHE["nc"]

    consts = _host_consts(inputs)
    vector = np.ascontiguousarray(np.asarray(inputs["vector"], np.float32))
    in_maps = []
    for c in range(N_CORES):
        m = dict(consts)
        m["x"] = np.ascontiguousarray(
            vector[c * BC:(c + 1) * BC].reshape(ROWS, CIN)
        )
        in_maps.append(m)

    res = run_bass_kernel_spmd(nc, in_maps, core_ids=list(range(N_CORES)))
    outs = [res.results[c]["out"].reshape(BC, J, CIN) for c in range(N_CORES)]
    return np.concatenate(outs, axis=0)


# revision 15
# speedup vs baseline: 1.0441x; 1.0441x over previous
"""Bone_Direction_GCN fused kernel, RPP=4 layout, for 8 Trainium2 NeuronCores.

Same math as kernel.py but each SBUF partition holds 4 CONSECUTIVE rows
(rows 4p+w, w in 0..3), so every HBM DMA descriptor covers 2 KB instead of
512 B (4x fewer descriptors; HW DMA is descriptor-fixed-cost bound).

The graph mix becomes 4 accumulating matmuls (one per row-in-partition slot w
of the source), x^T is recovered with 4 PE transposes, and the adj-mix runs
as 16 small matmuls whose host-built constants absorb the row permutation.
"""

import sys

sys.path.insert(0, "/opt/trn_rl_repo")

import numpy as np
import ml_dtypes

B, J, E = 16384, 17, 32
CIN, COUT = 128, 128
MID = COUT // 2
PROP = 0.5
SLOPE = 0.01

N_CORES = 8
BC = B // N_CORES          # batches per core (2048)
ROWS = BC * J              # rows per core (34816)
W = 4                      # rows per partition
P = 119                    # partitions used per macro-tile
RM = W * P                 # rows per macro-tile (476)
NB = RM // J               # batches per macro-tile (28)
LM = 4                     # macro-tiles per full DMA group
GSIZES = [4] * 18 + [1]    # 18 groups of 4 + final group of 1 (73 macros)
NG = len(GSIZES)
NMM = 73                   # all macro-tiles pipelined
NM = 73                    # total macro-tiles (73*476 = 34748)
GT = BC - NM * NB          # tail batches (4)
RT = GT * J                # tail rows (68)
R1 = 119                   # legacy sub-tile rows (epilogue/tail path)
PB = 120                   # padded transpose block (alignment)
CPAD = 256

assert NM * RM + RT == ROWS

_CACHE = {}


def _gcn_matrix(edge_index, edge_weight):
    row = edge_index[0].astype(np.int64)
    col = edge_index[1].astype(np.int64)
    loop = np.arange(J, dtype=np.int64)
    row_f = np.concatenate([row, loop])
    col_f = np.concatenate([col, loop])
    w_f = np.concatenate([edge_weight.astype(np.float32), np.ones(J, np.float32)])
    deg = np.zeros(J, np.float32)
    np.add.at(deg, col_f, w_f)
    safe = np.where(deg > 0, deg, 1.0).astype(np.float32)
    dis = np.where(deg > 0, 1.0 / np.sqrt(safe), 0.0).astype(np.float32)
    norm = dis[row_f] * w_f * dis[col_f]
    M = np.zeros((J, J), np.float32)
    np.add.at(M, (col_f, row_f), norm)
    return M


def _block_diag(block, n):
    j = block.shape[0]
    out = np.zeros((n * j, n * j), block.dtype)
    for g in range(n):
        out[g * j:(g + 1) * j, g * j:(g + 1) * j] = block
    return out


def _mix_consts_legacy(M, adj, g):
    r = g * J
    mix1 = _block_diag(M.T, g)
    mixI = np.concatenate([mix1, np.eye(r, dtype=np.float32)], axis=1)
    mix2 = _block_diag(PROP * adj, g)
    ones_row = np.ones((1, r), np.float32)
    s_row = np.tile(PROP * adj.sum(axis=0), g)[None, :]
    mix2e = np.concatenate([mix2, ones_row, s_row], axis=0)
    return mixI, mix2e


def _rpp4_consts(M, adj):
    """mixu [119, 4, 476]; m2 [121, 4, 4, 119] for the w/u-blocked adj mix."""
    Mblk = _block_diag(M.T, NB)                    # [476, 476]: rows=src, cols=dst
    mixu = np.zeros((P, W, RM), np.float32)
    for p in range(P):
        for u in range(W):
            mixu[p, u, :] = Mblk[W * p + u, :]
    adjm = PROP * adj                              # coeff d[v] -> out[w]: adjm[v, w]
    s_row_j = PROP * adj.sum(axis=0)               # [J]
    m2 = np.zeros((P + 2, W, W, P), np.float32)
    for pd in range(P):                            # y2e partition (d row 4*pd+u)
        for u in range(W):
            q = W * pd + u
            for po in range(P):                    # out partition (row 4*po+w)
                for w in range(W):
                    r = W * po + w
                    if q // J == r // J:
                        m2[pd, w, u, po] = adjm[q % J, r % J]
    for po in range(P):
        for w in range(W):
            r = W * po + w
            m2[P, w, 0, po] = 1.0                  # b1 row
            m2[P + 1, w, 0, po] = s_row_j[r % J]   # b4 row
    return mixu.reshape(P, W * RM), m2.reshape(P + 2, W * W * P)


def _build_bass(leaky_mode: str = "lrelu", **_ignored):
    import concourse.bacc as bacc
    import concourse.mybir as mybir
    import concourse.tile as tile
    from contextlib import ExitStack

    f32 = mybir.dt.float32
    bf16 = mybir.dt.bfloat16

    nc = bacc.Bacc("TRN2", target_bir_lowering=False, debug=False)

    x_d = nc.dram_tensor("x", [ROWS, CIN], f32, kind="ExternalInput").ap()
    mixu_d = nc.dram_tensor("mixu", [P, W * RM], bf16, kind="ExternalInput").ap()
    m2_d = nc.dram_tensor("m2", [P + 2, W * W * P], bf16, kind="ExternalInput").ap()
    ident_d = nc.dram_tensor("ident", [PB, PB], bf16, kind="ExternalInput").ap()
    mixIt_d = nc.dram_tensor("mixIt", [RT, CPAD], bf16, kind="ExternalInput").ap()
    mix2et_d = nc.dram_tensor("mix2et", [RT + 2, RT], bf16, kind="ExternalInput").ap()
    w1_d = nc.dram_tensor("w1", [CIN, COUT], bf16, kind="ExternalInput").ap()
    w2t_d = nc.dram_tensor("w2t", [CIN, MID], bf16, kind="ExternalInput").ap()
    w4t_d = nc.dram_tensor("w4t", [MID, COUT], bf16, kind="ExternalInput").ap()
    b2_d = nc.dram_tensor("b2", [MID, 1], f32, kind="ExternalInput").ap()
    ab2_d = nc.dram_tensor("ab2", [MID, 1], f32, kind="ExternalInput").ap()
    b1b4_d = nc.dram_tensor("b1b4", [2, W * COUT], bf16, kind="ExternalInput").ap()
    o_d = nc.dram_tensor("out", [ROWS, CIN], f32, kind="ExternalOutput").ap()

    with ExitStack() as ctx:
        tc = ctx.enter_context(tile.TileContext(nc))

        const = ctx.enter_context(tc.tile_pool(name="const", bufs=1))
        mixu_sb = const.tile_from(mixu_d)
        m2_sb = const.tile_from(m2_d)
        ident_sb = const.tile_from(ident_d)
        mixIt_sb = const.tile_from(mixIt_d)
        mix2et_sb = const.tile_from(mix2et_d)
        w1_sb = const.tile_from(w1_d)
        w2t_sb = const.tile_from(w2t_d)
        w4t_sb = const.tile_from(w4t_d)
        b2_sb = const.tile_from(b2_d)
        ab2_sb = const.tile_from(ab2_d)

        def leaky(hbf, psH):
            if leaky_mode == "lrelu":
                nc.scalar.activation(
                    hbf[:], psH[:],
                    func=mybir.ActivationFunctionType.Lrelu,
                    bias=b2_sb[:], scale=1.0, alpha=SLOPE,
                )
            else:
                a = lk_pool.tile(list(psH.shape), bf16, tag="lk_a", name="lk_a")
                nc.scalar.activation(
                    a[:], psH[:],
                    func=mybir.ActivationFunctionType.Identity,
                    bias=ab2_sb[:], scale=SLOPE,
                )
                nc.vector.scalar_tensor_tensor(
                    hbf[:], psH[:], b2_sb[:], a[:],
                    op0=mybir.AluOpType.add, op1=mybir.AluOpType.max,
                )

        y2e_pool = ctx.enter_context(tc.tile_pool(name="y2e", bufs=3))
        y2e_tiles = []
        for i in range(3):
            t = y2e_pool.tile([P + 2, W * COUT], bf16, tag=f"y2e{i}", name=f"y2e{i}")
            nc.sync.dma_start(out=t[P:P + 2, :], in_=b1b4_d)
            y2e_tiles.append(t)
        y2et_pool = ctx.enter_context(tc.tile_pool(name="y2et", bufs=1))
        y2et = y2et_pool.tile([RT + 2, COUT], bf16)
        nc.sync.dma_start(out=y2et[RT:RT + 2, :], in_=b1b4_d[:, 0:COUT])

        lk_pool = ctx.enter_context(tc.tile_pool(name="lk", bufs=2))

        xin_pool = ctx.enter_context(tc.tile_pool(name="xin", bufs=3))
        xbf_pool = ctx.enter_context(tc.tile_pool(name="xbf", bufs=3))
        xbf_ring = []
        for i in range(3):
            t = xbf_pool.tile([PB, W * CIN], bf16, tag=f"xbfr{i}", name=f"xbfr{i}")
            nc.gpsimd.memset(t[:], 0.0)
            xbf_ring.append(t)
        ogrp_pool = ctx.enter_context(tc.tile_pool(name="ogrp", bufs=3))
        xmxt_pool = ctx.enter_context(tc.tile_pool(name="xmxt", bufs=4))
        h_pool = ctx.enter_context(tc.tile_pool(name="h", bufs=3))

        psTm_pool = ctx.enter_context(tc.tile_pool(name="psTm", bufs=2, space="PSUM"))
        psTr_pool = ctx.enter_context(tc.tile_pool(name="psTr", bufs=2, space="PSUM"))
        psH_pool = ctx.enter_context(tc.tile_pool(name="psH", bufs=1, space="PSUM"))
        psY2_pool = ctx.enter_context(tc.tile_pool(name="psY2", bufs=1, space="PSUM"))
        psO_pool = ctx.enter_context(tc.tile_pool(name="psO", bufs=2, space="PSUM"))

        xin_tiles = [None] * NG
        ogrp_tiles = [None] * NG
        g_of, k_of, g_row = [], [], []
        r_acc = 0
        for g, sz in enumerate(GSIZES):
            g_row.append(r_acc)
            for k in range(sz):
                g_of.append(g)
                k_of.append(k)
            r_acc += sz * RM
        xbf_tiles = [None] * NMM
        psTm_tiles = [None] * NMM
        psTr_tiles = [None] * NMM
        xm_tiles = [None] * NMM
        xt_tiles = [None] * NMM
        psH_tiles = [None] * NMM
        psY2_tiles = [None] * NMM
        psO_tiles = [None] * NMM
        hbf_tiles = [None] * NMM

        MFREE = W * CIN            # 512 f32 per partition per macro

        def xin_slice(m):
            g, k = g_of[m], k_of[m]
            return xin_tiles[g][:, k * MFREE:(k + 1) * MFREE]

        def stage_load(g):
            r0 = g_row[g]
            sz = GSIZES[g]
            t = xin_pool.tile([P, sz * MFREE], f32, tag="xin", name="xin")
            xin_tiles[g] = t
            nc.sync.dma_start(
                out=t[:].rearrange("p (k q) -> p k q", q=MFREE),
                in_=x_d[r0:r0 + sz * RM, :].rearrange(
                    "(k p w) c -> p k (w c)", p=P, w=W),
            )
            ogrp_tiles[g] = ogrp_pool.tile(
                [P, sz * MFREE], f32, tag="ogrp", name="ogrp")

        def stage_cast(m):
            t = xbf_ring[m % 3]
            xbf_tiles[m] = t
            nc.gpsimd.tensor_copy(t[0:P, :], xin_slice(m))

        def stage_mix(m):
            xbf = xbf_tiles[m]
            psTm = psTm_pool.tile([CIN, RM], f32, tag="psTm", name="psTm")
            psTm_tiles[m] = psTm
            for u in range(W):
                nc.tensor.matmul(
                    psTm[:],
                    lhsT=xbf[0:P, u * CIN:(u + 1) * CIN],
                    rhs=mixu_sb[:].rearrange("p (u n) -> p u n", u=W)[:, u, :],
                    start=(u == 0), stop=(u == W - 1),
                )
            psTr = psTr_pool.tile([CIN, W * PB], bf16, tag="psTr", name="psTr")
            psTr_tiles[m] = psTr
            for u in range(W):
                nc.tensor.transpose(
                    psTr[:, u * PB:(u + 1) * PB],
                    in_=xbf[0:PB, u * CIN:(u + 1) * CIN],
                    identity=ident_sb[:],
                )

        def stage_copies(m):
            # xm: [128, (w p)] bf16 (w-major so W1 lhsT slices are contiguous)
            xm = xmxt_pool.tile([CIN, RM], bf16, tag="xm", name="xm")
            xm_tiles[m] = xm
            nc.vector.tensor_copy(
                xm[:].rearrange("c (w p) -> c w p", w=W),
                psTm_tiles[m][:].rearrange("c (p w) -> c w p", w=W),
            )
            xt = xmxt_pool.tile([CIN, W * PB], bf16, tag="xt", name="xt")
            xt_tiles[m] = xt
            nc.scalar.copy(xt[:], psTr_tiles[m][:])

        def stage_w2(m):
            psH = psH_pool.tile([MID, W * PB], f32, tag="psH", name="psH")
            psH_tiles[m] = psH
            nc.tensor.matmul(
                psH[:], lhsT=w2t_sb[:], rhs=xt_tiles[m][:], start=True, stop=True)
            hbf = h_pool.tile([MID, W * PB], bf16, tag="hbf", name="hbf")
            hbf_tiles[m] = hbf
            leaky(hbf, psH)

        def stage_w4(m):
            hbf = hbf_tiles[m]
            psY2 = psY2_pool.tile([P, W * COUT], f32, tag="psY2", name="psY2")
            psY2_tiles[m] = psY2
            for u in range(W):
                nc.tensor.matmul(
                    psY2[:, u * COUT:(u + 1) * COUT],
                    lhsT=hbf[:, u * PB:u * PB + P], rhs=w4t_sb[:],
                    start=True, stop=True,
                )

        def stage_y2e(m):
            psY2 = psY2_tiles[m]
            y2e = y2e_tiles[m % 3]
            nc.vector.tensor_copy(y2e[0:P, 0:2 * COUT], psY2[:, 0:2 * COUT])
            nc.scalar.copy(y2e[0:P, 2 * COUT:], psY2[:, 2 * COUT:])

        def stage_out(m):
            xm = xm_tiles[m]
            y2e = y2e_tiles[m % 3]
            m2v = m2_sb[:].rearrange("p (w u q) -> p w u q", w=W, u=W)
            psO = psO_pool.tile([P, W * COUT], f32, tag="psO", name="psO")
            psO_tiles[m] = psO
            for w in range(W):
                nc.tensor.matmul(
                    psO[:, w * COUT:(w + 1) * COUT],
                    lhsT=xm[:, w * P:(w + 1) * P], rhs=w1_sb[:],
                    start=True, stop=False, skip_group_check=True,
                )
                for u in range(W):
                    nc.tensor.matmul(
                        psO[:, w * COUT:(w + 1) * COUT],
                        lhsT=m2v[:, w, u, :],
                        rhs=y2e[:, u * COUT:(u + 1) * COUT],
                        start=False, stop=(u == W - 1), skip_group_check=True,
                    )

        def stage_add(m):
            g, k = g_of[m], k_of[m]
            nc.vector.tensor_add(
                ogrp_tiles[g][:, k * MFREE:(k + 1) * MFREE],
                psO_tiles[m][:], xin_slice(m))

        def stage_store(g):
            r0 = g_row[g]
            sz = GSIZES[g]
            nc.sync.dma_start(
                out=o_d[r0:r0 + sz * RM, :].rearrange(
                    "(k p w) c -> p k (w c)", p=P, w=W),
                in_=ogrp_tiles[g][:].rearrange("p (k q) -> p k q", q=MFREE),
            )

        for it in range(NMM + 3):
            m1, m2i, m3 = it - 1, it - 2, it - 3
            if it < NMM and k_of[it] == 0:
                stage_load(g_of[it])
            if it < NMM:
                stage_cast(it)
            if 0 <= m3 < NMM:
                stage_y2e(m3)
            if 0 <= m1 < NMM:
                stage_mix(m1)
            if 0 <= m2i < NMM:
                stage_w2(m2i)
            if 0 <= m1 < NMM:
                stage_copies(m1)
            if 0 <= m3 < NMM:
                stage_out(m3)
                stage_add(m3)
            if 0 <= m2i < NMM:
                stage_w4(m2i)
            if 0 <= m3 < NMM and k_of[m3] == GSIZES[g_of[m3]] - 1:
                stage_store(g_of[m3])

        # ---- tail (68 rows): legacy s-major path ----
        r0 = NM * RM
        xin = xin_pool.tile([RT, CIN], f32, tag="xin_t")
        nc.sync.dma_start(out=xin[:], in_=x_d[r0:r0 + RT, :])
        xbf = xbf_pool.tile([RT, CIN], bf16, tag="xbf_t")
        nc.gpsimd.tensor_copy(xbf[:], xin[:])
        psT = psTm_pool.tile([CIN, CPAD], f32, tag="psTm", name="psT_t")
        nc.tensor.matmul(psT[:], lhsT=xbf[:], rhs=mixIt_sb[:], start=True, stop=True)
        xm = xmxt_pool.tile([CIN, RT], bf16, tag="xm_t")
        nc.vector.tensor_copy(xm[:], psT[:, 0:RT])
        xt = xmxt_pool.tile([CIN, RT], bf16, tag="xt_t")
        nc.scalar.copy(xt[:], psT[:, RT:2 * RT])
        psH = psH_pool.tile([MID, RT], f32, tag="psH")
        nc.tensor.matmul(psH[:], lhsT=w2t_sb[:], rhs=xt[:], start=True, stop=True)
        hbf = h_pool.tile([MID, RT], bf16, tag="hbf")
        leaky(hbf, psH)
        psY2 = psY2_pool.tile([RT, COUT], f32, tag="psY2")
        nc.tensor.matmul(psY2[:], lhsT=hbf[:], rhs=w4t_sb[:], start=True, stop=True)
        nc.scalar.copy(y2et[0:RT, :], psY2[:])
        psO = psO_pool.tile([RT, COUT], f32, tag="psO")
        nc.tensor.matmul(psO[:], lhsT=xm[:], rhs=w1_sb[:],
                         start=True, stop=False, skip_group_check=True)
        nc.tensor.matmul(psO[:], lhsT=mix2et_sb[:], rhs=y2et[:],
                         start=False, stop=True, skip_group_check=True)
        out_sb = xin_pool.tile([RT, CIN], f32, tag="out_t")
        nc.vector.tensor_add(out_sb[:], psO[:], xin[:])
        nc.sync.dma_start(out=o_d[r0:r0 + RT, :], in_=out_sb[:])

    nc.compile()
    return nc


def _host_consts(inputs):
    bf = ml_dtypes.bfloat16
    M = _gcn_matrix(np.asarray(inputs["edge_index"]), np.asarray(inputs["edge_weight"]))
    adj = np.asarray(inputs["adj"], np.float32)
    mixu, m2 = _rpp4_consts(M, adj)
    mixIt, mix2et = _mix_consts_legacy(M, adj, GT)
    mixIt_p = np.zeros((RT, CPAD), np.float32)
    mixIt_p[:, :2 * RT] = mixIt
    W1 = np.asarray(inputs["W1"], np.float32)
    W2 = np.asarray(inputs["W2"], np.float32)
    W4 = np.asarray(inputs["W4"], np.float32)
    b1 = np.asarray(inputs["b1"], np.float32)
    b2 = np.asarray(inputs["b2"], np.float32)
    b4 = np.asarray(inputs["b4"], np.float32)
    b1b4 = np.stack([np.tile(b1, W), np.tile(b4, W)])
    return {
        "mixu": mixu.astype(bf),
        "m2": m2.astype(bf),
        "ident": np.eye(PB, dtype=np.float32).astype(bf),
        "mixIt": mixIt_p.astype(bf),
        "mix2et": mix2et.astype(bf),
        "w1": np.ascontiguousarray(W1).astype(bf),
        "w2t": np.ascontiguousarray(W2.T).astype(bf),
        "w4t": np.ascontiguousarray(W4.T).astype(bf),
        "b2": np.ascontiguousarray(b2[:, None]),
        "ab2": np.ascontiguousarray(SLOPE * b2[:, None]),
        "b1b4": b1b4.astype(bf),
    }


def kernel(**inputs) -> np.ndarray:
    from concourse.bass_utils import run_bass_kernel_spmd

    if "nc" not in _CACHE:
        _CACHE["nc"] = _build_bass()
    nc = _CACHE["nc"]

    consts = _host_consts(inputs)
    vector = np.ascontiguousarray(np.asarray(inputs["vector"], np.float32))
    in_maps = []
    for c in range(N_CORES):
        m = dict(consts)
        m["x"] = np.ascontiguousarray(
            vector[c * BC:(c + 1) * BC].reshape(ROWS, CIN)
        )
        in_maps.append(m)

    res = run_bass_kernel_spmd(nc, in_maps, core_ids=list(range(N_CORES)))
    outs = [res.results[c]["out"].reshape(BC, J, CIN) for c in range(N_CORES)]
    return np.concatenate(outs, axis=0)
